# revision 1
# baseline (speedup 1.0000x reference)
"""Trainium2 Bass kernel for nn_DenseAttentionOneHead (B=2, L=4096, H=1024).

Reference math:
    h   = hidden * cos + rotate_half(hidden) * sin      (RoPE)
    q   = h @ W_q.T
    out = (q @ h^T) @ h                                 (no softmax)

With no softmax the L x L score matrix factorizes away:
    out[b] = q[b] @ G[b],  G[b] = h[b].T @ h[b]  (H x H)
reducing the work from O(B L^2 H) to O(B L H^2) ~ 39 GFLOP total.

Sharding (8 NeuronCores): cores 0-3 own batch 0's four 1024-row L-chunks,
cores 4-7 batch 1. Each core computes a partial G over its chunk; one 4MB
AllReduce within each 4-core replica group (overlapped with the q^T
matmul) produces the full G. q^T comes from PE-transposed RoPE output so
no transposed operands are ever loaded from HBM. All matmuls run in
float32r (TF32-like ~13-bit mantissa, full PE rate at free-dim 512, rel
err ~1.5e-4); RoPE, PSUM accumulation and the AllReduce stay fp32.

Engine plan per core: DVE does RoPE + fp32r roundings; PE does 64
transposes (filling its stalls while RoPE streams) then 3 x 128 matmuls;
ACT does all PSUM->SBUF copies; the two HWDGE DMA rings are split (h/G
bounce/y on the SP ring; cos/sin/W_q/G-fetch on the ACT ring) so the h
stream and the G bounce-out are never queued behind other transfers.
"""

import os

import numpy as np

import jax

try:
    _cache_dir = os.path.join(os.path.expanduser("~"), ".cache", "bass_kernel_jax")
    os.makedirs(_cache_dir, exist_ok=True)
    jax.config.update("jax_compilation_cache_dir", _cache_dir)
    jax.config.update("jax_persistent_cache_min_compile_time_secs", 1.0)
except Exception:
    pass

import concourse.bacc as bacc
import concourse.mybir as mybir
import concourse.tile as tile
from concourse import masks
from concourse.bass_utils import run_bass_kernel_spmd

F32 = mybir.dt.float32
F32R = mybir.dt.float32r

B, L, H = 2, 4096, 1024
L_CHUNK = 1024
HH = H // 2
NT = L_CHUNK // 128
MT = H // 128
GROUPS = [[0, 1, 2, 3], [4, 5, 6, 7]]


def _emit_once(nc, tc, h_d, c1_d, s1_d, wqt_d, y_d):
    h_ap = h_d.ap().rearrange("(t p) c -> p t c", p=128)
    c1_ap = c1_d.ap().rearrange("(t p) c -> p t c", p=128)
    s1_ap = s1_d.ap().rearrange("(t p) c -> p t c", p=128)
    wqt_ap = wqt_d.ap().rearrange("(t p) c -> p t c", p=128)
    y_ap = y_d.ap().rearrange("(t p) c -> p t c", p=128)

    with (
        tc.tile_pool(name="persist", bufs=1) as persist,
        tc.tile_pool(name="stream", bufs=1) as stream,
        tc.tile_pool(name="psum", bufs=6, space="PSUM") as psum,
        tc.tile_pool(name="psum_t", bufs=2, space="PSUM") as psum_t,
        tc.tile_pool(name="dram", bufs=1, space="DRAM") as dram,
    ):
        hr = persist.tile([128, NT, H], F32R, name="hr")
        hrt = persist.tile([128, MT, L_CHUNK], F32R, name="hrt")
        wq_r = persist.tile([128, MT, H], F32R, name="wq_r")
        # qt reuses hr's memory: hr's last readers (G matmuls + transposes)
        # finish right before the qt copies start writing; Tile WAR deps
        # order them.
        qt = hr
        g_r = persist.tile([128, MT, H], F32R, name="g_r")

        ident_f = stream.tile([128, 128], F32, name="ident_f", tag="identf")
        masks.make_identity(nc, ident_f[:])
        ident = stream.tile([128, 128], F32R, name="ident", tag="ident")
        nc.vector.tensor_copy(ident[:], ident_f[:])

        # RoPE (DVE) + per-tile PE transposes
        for t in range(NT):
            ht = stream.tile([128, H], F32, name="ht", tag="ld1024", bufs=3)
            ct = stream.tile([128, HH], F32, name="ct", tag="ld512", bufs=4)
            st = stream.tile([128, HH], F32, name="st", tag="ld512", bufs=4)
            nc.sync.dma_start(ht[:], h_ap[:, t, :])
            nc.scalar.dma_start(ct[:], c1_ap[:, t, :])
            nc.scalar.dma_start(st[:], s1_ap[:, t, :])
            h1 = ht[:, 0:HH]
            h2 = ht[:, HH:H]
            m1 = stream.tile([128, HH], F32, name="m1", tag="tmp", bufs=4)
            m2 = stream.tile([128, HH], F32, name="m2", tag="tmp", bufs=4)
            nc.vector.tensor_mul(m1[:], h1, ct[:])
            nc.vector.tensor_mul(m2[:], h2, st[:])
            nc.vector.tensor_sub(hr[:, t, 0:HH], m1[:], m2[:])
            m3 = stream.tile([128, HH], F32, name="m3", tag="tmp", bufs=4)
            m4 = stream.tile([128, HH], F32, name="m4", tag="tmp", bufs=4)
            nc.vector.tensor_mul(m3[:], h2, ct[:])
            nc.vector.tensor_mul(m4[:], h1, st[:])
            nc.vector.tensor_add(hr[:, t, HH:H], m3[:], m4[:])
            for mt in range(MT):
                pst = psum_t.tile([128, 128], F32R, name="pst", tag="pst")
                nc.tensor.transpose(
                    pst[:], hr[:, t, mt * 128:(mt + 1) * 128], ident[:]
                )
                nc.scalar.copy(hrt[:, mt, t * 128:(t + 1) * 128], pst[:])

        # W_qT load (ACT ring, queued behind the c/s stream) + DVE round
        for mt in range(MT):
            wt = stream.tile([128, H], F32, name="wt", tag="ldw", bufs=2)
            nc.scalar.dma_start(wt[:], wqt_ap[:, mt, :])
            nc.vector.tensor_copy(wq_r[:, mt, :], wt[:])

        # G_part = hr.T @ hr; bounce per m-tile; one AllReduce over all of G
        bounce_in = dram.tile([128, MT * H], F32, name="bounce_in")
        bounce_out = dram.tile([128, MT * H], F32, name="bounce_out")
        b_in_t = bounce_in[:].rearrange("p (t c) -> p t c", t=MT)
        b_out_t = bounce_out[:].rearrange("p (t c) -> p t c", t=MT)
        for mt in range(MT):
            gb = stream.tile([128, H], F32, name="gb", tag="gb", bufs=2)
            for nh in range(2):
                ps = psum.tile([128, 512], F32, name="ps", tag="ps")
                for kt in range(NT):
                    nc.tensor.matmul(
                        ps[:],
                        hr[:, kt, mt * 128:(mt + 1) * 128],
                        hr[:, kt, nh * 512:(nh + 1) * 512],
                        start=(kt == 0),
                        stop=(kt == NT - 1),
                    )
                nc.scalar.copy(gb[:, nh * 512:(nh + 1) * 512], ps[:])
            nc.sync.dma_start(b_in_t[:, mt, :], gb[:])
        # single AllReduce: per-collective fixed cost dominates on this
        # fabric path, so one 4MB AR beats two pipelined 2MB ARs
        nc.gpsimd.collective_compute(
            "AllReduce",
            mybir.AluOpType.add,
            replica_groups=GROUPS,
            ins=[bounce_in[:]],
            outs=[bounce_out[:]],
        )

        # qt = (W_qT as weights) @ hrt
        for ot in range(MT):
            for lh in range(2):
                ps = psum.tile([128, 512], F32, name="ps", tag="ps")
                for kt in range(MT):
                    nc.tensor.matmul(
                        ps[:],
                        wq_r[:, kt, ot * 128:(ot + 1) * 128],
                        hrt[:, kt, lh * 512:(lh + 1) * 512],
                        start=(kt == 0),
                        stop=(kt == MT - 1),
                    )
                nc.scalar.copy(qt[:, ot, lh * 512:(lh + 1) * 512], ps[:])

        # fetch AllReduce result on the ACT ring; round to fp32r on DVE
        for mt in range(MT):
            gi = stream.tile([128, H], F32, name="gi", tag="gi", bufs=2)
            nc.scalar.dma_start(gi[:], b_out_t[:, mt, :])
            nc.vector.tensor_copy(g_r[:, mt, :], gi[:])

        # y = (qt as weights) @ G
        for lt in range(NT):
            for nh in range(2):
                ps = psum.tile([128, 512], F32, name="ps", tag="ps")
                for kt in range(MT):
                    nc.tensor.matmul(
                        ps[:],
                        qt[:, kt, lt * 128:(lt + 1) * 128],
                        g_r[:, kt, nh * 512:(nh + 1) * 512],
                        start=(kt == 0),
                        stop=(kt == MT - 1),
                    )
                ot = stream.tile([128, 512], F32, name="ot", tag="ld512", bufs=4)
                nc.scalar.copy(ot[:], ps[:])
                nc.sync.dma_start(y_ap[:, lt, nh * 512:(nh + 1) * 512], ot[:])


_NC_CACHE = {}


def _build():
    if "nc" in _NC_CACHE:
        return _NC_CACHE["nc"]
    nc = bacc.Bacc("TRN2", target_bir_lowering=False, debug=False, num_devices=8)
    h_d = nc.dram_tensor("h", [L_CHUNK, H], F32, kind="ExternalInput")
    c1_d = nc.dram_tensor("c1", [L_CHUNK, HH], F32, kind="ExternalInput")
    s1_d = nc.dram_tensor("s1", [L_CHUNK, HH], F32, kind="ExternalInput")
    wqt_d = nc.dram_tensor("wqt", [H, H], F32, kind="ExternalInput")
    y_d = nc.dram_tensor("y", [L_CHUNK, H], F32, kind="ExternalOutput")
    with tile.TileContext(nc) as tc:
        _emit_once(nc, tc, h_d, c1_d, s1_d, wqt_d, y_d)
    nc.compile()
    _NC_CACHE["nc"] = nc
    return nc


def kernel(hidden_states, W_q, cos, sin):
    hs = np.asarray(hidden_states, dtype=np.float32)
    wq = np.asarray(W_q, dtype=np.float32)
    cos = np.asarray(cos, dtype=np.float32)
    sin = np.asarray(sin, dtype=np.float32)
    wqt = np.ascontiguousarray(wq.T)
    in_maps = []
    for core in range(8):
        b, i = core // 4, core % 4
        sl = slice(i * L_CHUNK, (i + 1) * L_CHUNK)
        in_maps.append({
            "h": np.ascontiguousarray(hs[b, sl]),
            "c1": np.ascontiguousarray(cos[sl, :HH]),
            "s1": np.ascontiguousarray(sin[sl, :HH]),
            "wqt": wqt,
        })

    nc = _build()
    res = run_bass_kernel_spmd(nc, in_maps, core_ids=list(range(8)))

    out = np.empty((B, L, H), dtype=np.float32)
    for core, r in enumerate(res.results):
        b, i = core // 4, core % 4
        out[b, i * L_CHUNK:(i + 1) * L_CHUNK] = r["y"]
    return out



# revision 11
# speedup vs baseline: 2.6533x; 2.6533x over previous
"""Trainium2 Bass kernel for nn_DenseAttentionOneHead (B=2, L=4096, H=1024).

Reference math:
    h   = hidden * cos + rotate_half(hidden) * sin      (RoPE)
    q   = h @ W_q.T
    out = (q @ h^T) @ h                                 (no softmax)

With no softmax the L x L score matrix factorizes away:
    out[b] = h[b] @ M[b],   M[b] = W_q^T @ G[b],   G[b] = h[b].T @ h[b]

Sharding (8 NeuronCores): cores 0-3 own batch 0, cores 4-7 batch 1. Core
rank r computes a partial G over its 1024-row L-chunk, then ONE bf16
ReduceScatter (out 0.5MB/core, vs the 4MB fp32 AllReduce of the previous
version) hands it the summed 256-row stripe G[js,:] -- which by symmetry
of G is the column stripe G[:,js]^T. It computes M[:,js] = W_q^T G[:,js]
(1/4 of M) and then the OUTPUT COLUMN STRIPE for the whole batch,
y[:, js] = h~ @ M[:, js], so no AllGather is ever needed; the host
concatenates the 8 stripes.

All matmul operands are bf16 (PSUM accumulation fp32; tolerance is 2e-2,
measured ~1e-3). h~^T for the y matmul is produced by running RoPE
directly on host-transposed inputs on the DVE (pairing k and k+512 is
tile-wise elementwise in the transposed layout), which removes all 256
PE transposes and their PSUM drains. G drains are split ACT/GpSimd so
the ReduceScatter starts early; the lower-left quarter of G is
reconstructed by PE-transposing the upper-right (G is symmetric), saving
a quarter of the G matmul work.
"""

import os

import numpy as np

import jax

try:
    _cache_dir = os.path.join(os.path.expanduser("~"), ".cache", "bass_kernel_jax")
    os.makedirs(_cache_dir, exist_ok=True)
    jax.config.update("jax_compilation_cache_dir", _cache_dir)
    jax.config.update("jax_persistent_cache_min_compile_time_secs", 1.0)
except Exception:
    pass

import ml_dtypes

import concourse.bacc as bacc
import concourse.mybir as mybir
import concourse.tile as tile
from concourse import masks
from concourse.bass_utils import run_bass_kernel_spmd

F32 = mybir.dt.float32
BF16 = mybir.dt.bfloat16
BF16_NP = ml_dtypes.bfloat16

B, L, H = 2, 4096, 1024
HH = H // 2          # 512
LC = 1024            # own L-chunk rows per core
JS = 256             # output column stripe per core
NT = LC // 128       # 8 tiles in own chunk
LT = L // 128        # 32 l-tiles in full batch
MT = H // 128        # 8 tiles across H
GROUPS = [[0, 1, 2, 3], [4, 5, 6, 7]]


def _emit_once(nc, tc, hn_d, c1_d, s1_d, ht_d, ct_d, st_d, wq_d, y_d, dbg=None):
    hn_ap = hn_d.ap().rearrange("(t p) c -> p t c", p=128)   # [128, 8, 1024]
    c1_ap = c1_d.ap().rearrange("(t p) c -> p t c", p=128)   # [128, 8, 512]
    s1_ap = s1_d.ap().rearrange("(t p) c -> p t c", p=128)
    ht_ap = ht_d.ap().rearrange("(t p) c -> p t c", p=128)   # [128, 8, 4096]
    ct_ap = ct_d.ap().rearrange("(t p) c -> p t c", p=128)   # [128, 4, 4096]
    st_ap = st_d.ap().rearrange("(t p) c -> p t c", p=128)
    wq_ap = wq_d.ap().rearrange("(t p) c -> p t c", p=128)   # [128, 8, 1024]
    y_ap = y_d.ap().rearrange("(t p) c -> p t c", p=128)     # [128, 32, 256]

    with (
        tc.tile_pool(name="persist", bufs=1) as persist,
        tc.tile_pool(name="stream", bufs=1) as stream,
        tc.tile_pool(name="psum", bufs=4, space="PSUM") as psum,
        tc.tile_pool(name="psum2", bufs=4, space="PSUM") as psum2,
        tc.tile_pool(name="dram", bufs=1, space="DRAM") as dram,
    ):
        hr = persist.tile([128, NT, H], BF16, name="hr")        # own RoPE'd
        hTr = persist.tile([128, MT, L], BF16, name="hTr")      # full h~^T
        wqsb = persist.tile([128, MT, H], BF16, name="wqsb")
        gL = persist.tile([128, 4, H], BF16, name="gL")         # G rows mt0-3
        gt = persist.tile([128, MT, JS], BF16, name="gt")       # G[:,js] col stripe
        msb = persist.tile([128, MT, JS], BF16, name="msb")     # M[:,js]

        ident_f = stream.tile([128, 128], F32, name="ident_f", tag="identf")
        masks.make_identity(nc, ident_f[:])
        ident = stream.tile([128, 128], BF16, name="ident", tag="ident")
        nc.vector.tensor_copy(ident[:], ident_f[:])

        gbounce = dram.tile([LC, H], BF16, name="gbounce")
        rsout = dram.tile([JS, H], BF16, name="rsout")
        gb_ap = gbounce[:].rearrange("(t p) c -> p t c", p=128)  # [128, 8, 1024]
        rs_ap = rsout[:].rearrange("(t p) c -> p t c", p=128)    # [128, 2, 1024]

        # ---- Phase 1: own natural chunk stream + RoPE (DVE), ring A ----
        for t in range(NT):
            hnt = stream.tile([128, H], BF16, name="hnt", tag="hnt", bufs=3)
            c1t = stream.tile([128, HH], BF16, name="c1t", tag="cst", bufs=6)
            s1t = stream.tile([128, HH], BF16, name="s1t", tag="cst", bufs=6)
            nc.sync.dma_start(hnt[:], hn_ap[:, t, :])
            nc.sync.dma_start(c1t[:], c1_ap[:, t, :])
            nc.sync.dma_start(s1t[:], s1_ap[:, t, :])
            m1 = stream.tile([128, HH], BF16, name="m1", tag="tmp", bufs=8)
            m2 = stream.tile([128, HH], BF16, name="m2", tag="tmp", bufs=8)
            nc.vector.tensor_mul(m1[:], hnt[:, 0:HH], c1t[:])
            nc.vector.tensor_mul(m2[:], hnt[:, HH:H], s1t[:])
            nc.vector.tensor_sub(hr[:, t, 0:HH], m1[:], m2[:])
            m3 = stream.tile([128, HH], BF16, name="m3", tag="tmp", bufs=8)
            m4 = stream.tile([128, HH], BF16, name="m4", tag="tmp", bufs=8)
            nc.vector.tensor_mul(m3[:], hnt[:, HH:H], c1t[:])
            nc.vector.tensor_mul(m4[:], hnt[:, 0:HH], s1t[:])
            nc.vector.tensor_add(hr[:, t, HH:H], m3[:], m4[:])

        # W_q load on ring A after the own stream (FIFO keeps it off the
        # critical first 12us), before the big h^T stream.
        for t in range(MT):
            nc.sync.dma_start(wqsb[:, t, :], wq_ap[:, t, :])

        # ---- Phase 2: G rows mt0-3, full width (kt-outer: pipelines with
        # the RoPE stream). 8 chains = 8 PSUM banks (4 from each pool). ----
        ps_gl = []
        for i in range(8):
            pool = psum if i < 4 else psum2
            pgl = pool.tile([128, 512], F32, name=f"pgl{i}", tag="ps" if i < 4 else "mchain")
            ps_gl.append(pgl)
        for kt in range(NT):
            for mt in range(4):
                for nh in range(2):
                    nc.tensor.matmul(
                        ps_gl[mt * 2 + nh][:],
                        hr[:, kt, mt * 128:(mt + 1) * 128],
                        hr[:, kt, nh * 512:(nh + 1) * 512],
                        start=(kt == 0),
                        stop=(kt == NT - 1),
                    )
        # drains split ACT / GpSimd; bounce per row-tile on the GpSimd ring
        for mt in range(4):
            nc.scalar.copy(gL[:, mt, 0:512], ps_gl[mt * 2][:])
            nc.scalar.copy(gL[:, mt, 512:1024], ps_gl[mt * 2 + 1][:])
            nc.gpsimd.dma_start(gb_ap[:, mt, :], gL[:, mt, :])

        # ---- Phase 3: G rows mt4-7. Right half (cols 512:1024) by matmul;
        # left half = transpose of gL[:, 0:4, 512:1024] (G symmetric). ----
        ps_gr = [psum.tile([128, 512], F32, name=f"pgr{i}", tag="ps") for i in range(4)]
        for kt in range(NT):
            for mt in range(4, 8):
                nc.tensor.matmul(
                    ps_gr[mt - 4][:],
                    hr[:, kt, mt * 128:(mt + 1) * 128],
                    hr[:, kt, 512:1024],
                    start=(kt == 0),
                    stop=(kt == NT - 1),
                )
        ps_rc = [psum2.tile([128, 512], BF16, name=f"prc{i}", tag="mchain") for i in range(4)]
        for mt in range(4, 8):
            for nt in range(4):
                nc.tensor.transpose(
                    ps_rc[mt - 4][:, nt * 128:(nt + 1) * 128],
                    gL[:, nt, mt * 128:(mt + 1) * 128],
                    ident[:],
                )
        for mt in range(4, 8):
            gf = stream.tile([128, H], BF16, name="gf", tag="gf", bufs=2)
            nc.scalar.copy(gf[:, 0:512], ps_rc[mt - 4][:])
            nc.scalar.copy(gf[:, 512:1024], ps_gr[mt - 4][:])
            nc.gpsimd.dma_start(gb_ap[:, mt, :], gf[:])

        # ---- Phase 4: ReduceScatter (bf16, out = 0.5MB stripe) ----
        nc.gpsimd.collective_compute(
            "ReduceScatter",
            mybir.AluOpType.add,
            replica_groups=GROUPS,
            ins=[gbounce[:]],
            outs=[rsout[:]],
        )

        # ---- Phase 7 (emitted here so ring A FIFO = own, wq, h^T stream;
        # DVE queue = own RoPE then these): RoPE on transposed layout.
        # Pairs (t, t+4) are tile-wise elementwise in k; 2-wide ops. ----
        for blk in range(4):
            cs = slice(blk * 1024, (blk + 1) * 1024)
            for g in range(2):
                ta = slice(2 * g, 2 * g + 2)
                tb = slice(2 * g + 4, 2 * g + 6)
                h2a = stream.tile([128, 2, 1024], BF16, name="h2a", tag="ht2", bufs=3)
                h2b = stream.tile([128, 2, 1024], BF16, name="h2b", tag="ht2", bufs=3)
                c2 = stream.tile([128, 2, 1024], BF16, name="c2", tag="cs2", bufs=2)
                s2 = stream.tile([128, 2, 1024], BF16, name="s2", tag="cs2b", bufs=2)
                nc.sync.dma_start(h2a[:], ht_ap[:, ta, cs])
                nc.sync.dma_start(h2b[:], ht_ap[:, tb, cs])
                nc.sync.dma_start(c2[:], ct_ap[:, ta, cs])
                nc.sync.dma_start(s2[:], st_ap[:, ta, cs])
                ma = stream.tile([128, 2, 1024], BF16, name="ma", tag="tmp2", bufs=6)
                mb = stream.tile([128, 2, 1024], BF16, name="mb", tag="tmp2", bufs=6)
                nc.vector.tensor_mul(ma[:], h2a[:], c2[:])
                nc.vector.tensor_mul(mb[:], h2b[:], s2[:])
                nc.vector.tensor_sub(hTr[:, ta, cs], ma[:], mb[:])
                mc = stream.tile([128, 2, 1024], BF16, name="mc", tag="tmp2", bufs=6)
                md = stream.tile([128, 2, 1024], BF16, name="md", tag="tmp2", bufs=6)
                nc.vector.tensor_mul(mc[:], h2b[:], c2[:])
                nc.vector.tensor_mul(md[:], h2a[:], s2[:])
                nc.vector.tensor_add(hTr[:, tb, cs], mc[:], md[:])

        # ---- Phase 5/6: stripe fetch, transpose to G[:,js], M chains.
        # M[i, js] = sum_k W_q[k, i] G[k, js]; kt-pipelined after each
        # stripe-transpose drain. 8 it-chains packed 2-per-bank. ----
        s0 = stream.tile([128, H], BF16, name="s0", tag="s0")
        s1t_ = stream.tile([128, H], BF16, name="s1t_", tag="s1")
        nc.scalar.dma_start(s0[:], rs_ap[:, 0, :])
        nc.scalar.dma_start(s1t_[:], rs_ap[:, 1, :])
        for kt in range(MT):
            pst = psum.tile([128, 512], BF16, name="pst", tag="ps", bufs=4)
            nc.tensor.transpose(pst[:, 0:128], s0[:, kt * 128:(kt + 1) * 128], ident[:])
            nc.tensor.transpose(pst[:, 128:256], s1t_[:, kt * 128:(kt + 1) * 128], ident[:])
            nc.scalar.copy(gt[:, kt, :], pst[:, 0:256])
        for it in range(MT):
            pm = psum2.tile([128, 256], F32, name="pm", tag="mchain", bufs=4)
            for kt in range(MT):
                nc.tensor.matmul(
                    pm[:],
                    wqsb[:, kt, it * 128:(it + 1) * 128],
                    gt[:, kt, :],
                    start=(kt == 0),
                    stop=(kt == MT - 1),
                )
            nc.scalar.copy(msb[:, it, :], pm[:])

        # ---- Phase 8: y[:, js] = h~ @ M[:, js], 32 l-tiles (2 per bank) ----
        for lt in range(LT):
            py = psum.tile([128, 256], F32, name="py", tag="ps", bufs=4)
            for kt in range(MT):
                nc.tensor.matmul(
                    py[:],
                    hTr[:, kt, lt * 128:(lt + 1) * 128],
                    msb[:, kt, :],
                    start=(kt == 0),
                    stop=(kt == MT - 1),
                )
            yo = stream.tile([128, 256], BF16, name="yo", tag="yo", bufs=4)
            nc.scalar.copy(yo[:], py[:])
            nc.scalar.dma_start(y_ap[:, lt, :], yo[:])

        if dbg is not None:
            aps = {k: d.ap().rearrange("(t p) c -> p t c", p=128)
                   for k, d in dbg.items()}
            for t in range(NT):
                nc.sync.dma_start(aps["dbg_hr"][:, t, :], hr[:, t, :])
            for t in range(MT):
                nc.sync.dma_start(aps["dbg_htr"][:, t, :], hTr[:, t, :])
                nc.sync.dma_start(aps["dbg_gt"][:, t, :], gt[:, t, :])
                nc.sync.dma_start(aps["dbg_m"][:, t, :], msb[:, t, :])
            nc.sync.dma_start(aps["dbg_s"][:, 0, :], s0[:])
            nc.sync.dma_start(aps["dbg_s"][:, 1, :], s1t_[:])
            for t in range(NT):
                gbt = stream.tile([128, H], BF16, name="gbt", tag="gbt", bufs=2)
                nc.scalar.dma_start(gbt[:], gb_ap[:, t, :])
                nc.vector.tensor_copy(gbt[:], gbt[:])
                nc.sync.dma_start(aps["dbg_gb"][:, t, :], gbt[:])


_NC_CACHE = {}


def _build():
    if "nc" in _NC_CACHE:
        return _NC_CACHE["nc"]
    nc = bacc.Bacc("TRN2", target_bir_lowering=False, debug=False, num_devices=8)
    hn_d = nc.dram_tensor("hn", [LC, H], BF16, kind="ExternalInput")
    c1_d = nc.dram_tensor("c1", [LC, HH], BF16, kind="ExternalInput")
    s1_d = nc.dram_tensor("s1", [LC, HH], BF16, kind="ExternalInput")
    ht_d = nc.dram_tensor("ht", [H, L], BF16, kind="ExternalInput")
    ct_d = nc.dram_tensor("ct", [HH, L], BF16, kind="ExternalInput")
    st_d = nc.dram_tensor("st", [HH, L], BF16, kind="ExternalInput")
    wq_d = nc.dram_tensor("wq", [H, H], BF16, kind="ExternalInput")
    y_d = nc.dram_tensor("y", [L, JS], BF16, kind="ExternalOutput")
    with tile.TileContext(nc) as tc:
        _emit_once(nc, tc, hn_d, c1_d, s1_d, ht_d, ct_d, st_d, wq_d, y_d)
    nc.compile()
    _NC_CACHE["nc"] = nc
    return nc


def _in_maps(hidden_states, W_q, cos, sin):
    hs = np.asarray(hidden_states, dtype=np.float32)
    wq = np.asarray(W_q, dtype=np.float32).astype(BF16_NP)
    cos = np.asarray(cos, dtype=np.float32)
    sin = np.asarray(sin, dtype=np.float32)
    c1 = cos[:, :HH].astype(BF16_NP)
    s1 = sin[:, :HH].astype(BF16_NP)
    ct = np.ascontiguousarray(c1.T)
    st = np.ascontiguousarray(s1.T)
    hsb = hs.astype(BF16_NP)
    hts = [np.ascontiguousarray(hsb[b].T) for b in range(B)]
    maps = []
    for core in range(8):
        b, r = core // 4, core % 4
        sl = slice(r * LC, (r + 1) * LC)
        maps.append({
            "hn": np.ascontiguousarray(hsb[b, sl]),
            "c1": np.ascontiguousarray(c1[sl]),
            "s1": np.ascontiguousarray(s1[sl]),
            "ht": hts[b],
            "ct": ct,
            "st": st,
            "wq": wq,
        })
    return maps


def kernel(hidden_states, W_q, cos, sin):
    maps = _in_maps(hidden_states, W_q, cos, sin)
    nc = _build()
    res = run_bass_kernel_spmd(nc, maps, core_ids=list(range(8)))
    out = np.empty((B, L, H), dtype=np.float32)
    for core, r in enumerate(res.results):
        b, rr = core // 4, core % 4
        out[b, :, rr * JS:(rr + 1) * JS] = np.asarray(r["y"], dtype=np.float32)
    return out


# revision 15
# speedup vs baseline: 2.6702x; 1.0064x over previous
"""Trainium2 Bass kernel for nn_DenseAttentionOneHead (B=2, L=4096, H=1024).

Reference math:
    h   = hidden * cos + rotate_half(hidden) * sin      (RoPE)
    q   = h @ W_q.T
    out = (q @ h^T) @ h                                 (no softmax)

With no softmax the L x L score matrix factorizes away:
    out[b] = h[b] @ M[b],   M[b] = W_q^T @ G[b],   G[b] = h[b].T @ h[b]

Sharding (8 NeuronCores): cores 0-3 own batch 0, cores 4-7 batch 1. Core
rank r computes a partial G over its 1024-row L-chunk, then ONE bf16
ReduceScatter (out 0.5MB/core, vs the 4MB fp32 AllReduce of the previous
version) hands it the summed 256-row stripe G[js,:] -- which by symmetry
of G is the column stripe G[:,js]^T. It computes M[:,js] = W_q^T G[:,js]
(1/4 of M) and then the OUTPUT COLUMN STRIPE for the whole batch,
y[:, js] = h~ @ M[:, js], so no AllGather is ever needed; the host
concatenates the 8 stripes.

All matmul operands are bf16 (PSUM accumulation fp32; tolerance is 2e-2,
measured ~1e-3). h~^T for the y matmul is produced by running RoPE
directly on host-transposed inputs on the DVE (pairing k and k+512 is
tile-wise elementwise in the transposed layout), which removes all 256
PE transposes and their PSUM drains. G drains are split ACT/GpSimd so
the ReduceScatter starts early; the lower-left quarter of G is
reconstructed by PE-transposing the upper-right (G is symmetric), saving
a quarter of the G matmul work.
"""

import os

import numpy as np

import jax

try:
    _cache_dir = os.path.join(os.path.expanduser("~"), ".cache", "bass_kernel_jax")
    os.makedirs(_cache_dir, exist_ok=True)
    jax.config.update("jax_compilation_cache_dir", _cache_dir)
    jax.config.update("jax_persistent_cache_min_compile_time_secs", 1.0)
except Exception:
    pass

import ml_dtypes

import concourse.bacc as bacc
import concourse.mybir as mybir
import concourse.tile as tile
from concourse import masks
from concourse.bass_utils import run_bass_kernel_spmd

F32 = mybir.dt.float32
BF16 = mybir.dt.bfloat16
BF16_NP = ml_dtypes.bfloat16

B, L, H = 2, 4096, 1024
HH = H // 2          # 512
LC = 1024            # own L-chunk rows per core
JS = 256             # output column stripe per core
NT = LC // 128       # 8 tiles in own chunk
LT = L // 128        # 32 l-tiles in full batch
MT = H // 128        # 8 tiles across H
GROUPS = [[0, 1, 2, 3], [4, 5, 6, 7]]


def _emit_once(nc, tc, hn_d, c1_d, s1_d, ht_d, ct_d, st_d, wq_d, y_d, dbg=None):
    hn_ap = hn_d.ap().rearrange("(t p) c -> p t c", p=128)   # [128, 8, 1024]
    c1_ap = c1_d.ap().rearrange("(t p) c -> p t c", p=128)   # [128, 8, 512]
    s1_ap = s1_d.ap().rearrange("(t p) c -> p t c", p=128)
    ht_ap = ht_d.ap().rearrange("(t p) c -> p t c", p=128)   # [128, 8, 4096]
    ct_ap = ct_d.ap().rearrange("(t p) c -> p t c", p=128)   # [128, 4, 4096]
    st_ap = st_d.ap().rearrange("(t p) c -> p t c", p=128)
    wq_ap = wq_d.ap().rearrange("(t p) c -> p t c", p=128)   # [128, 8, 1024]
    y_ap = y_d.ap().rearrange("(t p) c -> p t c", p=128)     # [128, 32, 256]

    with (
        tc.tile_pool(name="persist", bufs=1) as persist,
        tc.tile_pool(name="stream", bufs=1) as stream,
        tc.tile_pool(name="psum", bufs=4, space="PSUM") as psum,
        tc.tile_pool(name="psum2", bufs=4, space="PSUM") as psum2,
        tc.tile_pool(name="dram", bufs=1, space="DRAM") as dram,
    ):
        hr = persist.tile([128, NT, H], BF16, name="hr")        # own RoPE'd
        hTr = persist.tile([128, MT, L], BF16, name="hTr")      # full h~^T
        wqsb = persist.tile([128, MT, H], BF16, name="wqsb")
        gL = persist.tile([128, 4, H], BF16, name="gL")         # G rows mt0-3
        gt = persist.tile([128, MT, JS], BF16, name="gt")       # G[:,js] col stripe
        msb = persist.tile([128, MT, JS], BF16, name="msb")     # M[:,js]

        ident_f = stream.tile([128, 128], F32, name="ident_f", tag="identf")
        masks.make_identity(nc, ident_f[:])
        ident = stream.tile([128, 128], BF16, name="ident", tag="ident")
        nc.vector.tensor_copy(ident[:], ident_f[:])

        gbounce = dram.tile([LC, H], BF16, name="gbounce")
        rsout = dram.tile([JS, H], BF16, name="rsout")
        gb_ap = gbounce[:].rearrange("(t p) c -> p t c", p=128)  # [128, 8, 1024]
        rs_ap = rsout[:].rearrange("(t p) c -> p t c", p=128)    # [128, 2, 1024]

        # ---- Phase 1: own natural chunk stream + RoPE (DVE), ring A ----
        for t in range(NT):
            hnt = stream.tile([128, H], BF16, name="hnt", tag="hnt", bufs=3)
            c1t = stream.tile([128, HH], BF16, name="c1t", tag="cst", bufs=6)
            s1t = stream.tile([128, HH], BF16, name="s1t", tag="cst", bufs=6)
            nc.sync.dma_start(hnt[:], hn_ap[:, t, :])
            nc.sync.dma_start(c1t[:], c1_ap[:, t, :])
            nc.sync.dma_start(s1t[:], s1_ap[:, t, :])
            m1 = stream.tile([128, HH], BF16, name="m1", tag="tmp", bufs=8)
            m2 = stream.tile([128, HH], BF16, name="m2", tag="tmp", bufs=8)
            nc.vector.tensor_mul(m1[:], hnt[:, 0:HH], c1t[:])
            nc.vector.tensor_mul(m2[:], hnt[:, HH:H], s1t[:])
            nc.vector.tensor_sub(hr[:, t, 0:HH], m1[:], m2[:])
            m3 = stream.tile([128, HH], BF16, name="m3", tag="tmp", bufs=8)
            m4 = stream.tile([128, HH], BF16, name="m4", tag="tmp", bufs=8)
            nc.vector.tensor_mul(m3[:], hnt[:, HH:H], c1t[:])
            nc.vector.tensor_mul(m4[:], hnt[:, 0:HH], s1t[:])
            nc.vector.tensor_add(hr[:, t, HH:H], m3[:], m4[:])

        # W_q load on ring A after the own stream (FIFO keeps it off the
        # critical first 12us), before the big h^T stream.
        for t in range(MT):
            nc.sync.dma_start(wqsb[:, t, :], wq_ap[:, t, :])

        # ---- Phase 2: G rows mt0-3, full width (kt-outer: pipelines with
        # the RoPE stream). 8 chains = 8 PSUM banks (4 from each pool). ----
        ps_gl = []
        for i in range(8):
            pool = psum if i < 4 else psum2
            pgl = pool.tile([128, 512], F32, name=f"pgl{i}", tag="ps" if i < 4 else "mchain")
            ps_gl.append(pgl)
        for kt in range(NT):
            for mt in range(4):
                for nh in range(2):
                    nc.tensor.matmul(
                        ps_gl[mt * 2 + nh][:],
                        hr[:, kt, mt * 128:(mt + 1) * 128],
                        hr[:, kt, nh * 512:(nh + 1) * 512],
                        start=(kt == 0),
                        stop=(kt == NT - 1),
                    )
        # drains split ACT (cols 0:512) / DVE (cols 512:1024) so the bounce
        # leaves ~7us earlier; bounce per row-tile on the GpSimd ring
        for mt in range(4):
            nc.scalar.copy(gL[:, mt, 0:512], ps_gl[mt * 2][:])
            nc.vector.tensor_copy(gL[:, mt, 512:1024], ps_gl[mt * 2 + 1][:])
            nc.gpsimd.dma_start(gb_ap[:, mt, :], gL[:, mt, :])

        # ---- Phase 3: G rows mt4-7. Right half (cols 512:1024) by matmul;
        # left half = transpose of gL[:, 0:4, 512:1024] (G symmetric). ----
        ps_gr = [psum.tile([128, 512], F32, name=f"pgr{i}", tag="ps") for i in range(4)]
        for kt in range(NT):
            for mt in range(4, 8):
                nc.tensor.matmul(
                    ps_gr[mt - 4][:],
                    hr[:, kt, mt * 128:(mt + 1) * 128],
                    hr[:, kt, 512:1024],
                    start=(kt == 0),
                    stop=(kt == NT - 1),
                )
        ps_rc = [psum2.tile([128, 512], BF16, name=f"prc{i}", tag="mchain") for i in range(4)]
        for mt in range(4, 8):
            for nt in range(4):
                nc.tensor.transpose(
                    ps_rc[mt - 4][:, nt * 128:(nt + 1) * 128],
                    gL[:, nt, mt * 128:(mt + 1) * 128],
                    ident[:],
                )
        for mt in range(4, 8):
            gf = stream.tile([128, H], BF16, name="gf", tag="gf", bufs=4)
            nc.scalar.copy(gf[:, 512:1024], ps_gr[mt - 4][:])
            nc.scalar.copy(gf[:, 0:512], ps_rc[mt - 4][:])
            nc.gpsimd.dma_start(gb_ap[:, mt, :], gf[:])

        # ---- Phase 4: ReduceScatter (bf16, out = 0.5MB stripe) ----
        nc.gpsimd.collective_compute(
            "ReduceScatter",
            mybir.AluOpType.add,
            replica_groups=GROUPS,
            ins=[gbounce[:]],
            outs=[rsout[:]],
        )

        # ---- Phase 7 (emitted here so ring A FIFO = own, wq, h^T stream;
        # DVE queue = own RoPE then these): RoPE on transposed layout.
        # Pairs (t, t+4) are tile-wise elementwise in k; 2-wide ops. ----
        for blk in range(4):
            cs = slice(blk * 1024, (blk + 1) * 1024)
            for g in range(2):
                ta = slice(2 * g, 2 * g + 2)
                tb = slice(2 * g + 4, 2 * g + 6)
                h2a = stream.tile([128, 2, 1024], BF16, name="h2a", tag="ht2", bufs=3)
                h2b = stream.tile([128, 2, 1024], BF16, name="h2b", tag="ht2", bufs=3)
                c2 = stream.tile([128, 2, 1024], BF16, name="c2", tag="cs2", bufs=2)
                s2 = stream.tile([128, 2, 1024], BF16, name="s2", tag="cs2b", bufs=2)
                for i in range(2):
                    nc.sync.dma_start(h2a[:, i, :], ht_ap[:, 2 * g + i, cs])
                    nc.sync.dma_start(h2b[:, i, :], ht_ap[:, 2 * g + 4 + i, cs])
                    nc.sync.dma_start(c2[:, i, :], ct_ap[:, 2 * g + i, cs])
                    nc.sync.dma_start(s2[:, i, :], st_ap[:, 2 * g + i, cs])
                ma = stream.tile([128, 2, 1024], BF16, name="ma", tag="tmp2", bufs=6)
                mb = stream.tile([128, 2, 1024], BF16, name="mb", tag="tmp2", bufs=6)
                nc.vector.tensor_mul(ma[:], h2a[:], c2[:])
                nc.vector.tensor_mul(mb[:], h2b[:], s2[:])
                nc.vector.tensor_sub(hTr[:, ta, cs], ma[:], mb[:])
                mc = stream.tile([128, 2, 1024], BF16, name="mc", tag="tmp2", bufs=6)
                md = stream.tile([128, 2, 1024], BF16, name="md", tag="tmp2", bufs=6)
                nc.vector.tensor_mul(mc[:], h2b[:], c2[:])
                nc.vector.tensor_mul(md[:], h2a[:], s2[:])
                nc.vector.tensor_add(hTr[:, tb, cs], mc[:], md[:])

        # ---- Phase 5/6: stripe fetch, transpose to G[:,js], M chains.
        # M[i, js] = sum_k W_q[k, i] G[k, js]; kt-pipelined after each
        # stripe-transpose drain. 8 it-chains packed 2-per-bank. ----
        s0 = stream.tile([128, H], BF16, name="s0", tag="s0")
        s1t_ = stream.tile([128, H], BF16, name="s1t_", tag="s1")
        nc.scalar.dma_start(s0[:], rs_ap[:, 0, :])
        nc.scalar.dma_start(s1t_[:], rs_ap[:, 1, :])
        # Transposes paired 2-kt per PSUM bank, drained as one [128,512] copy;
        # M chains for it 0-3 run kt-outer, consuming each gt pair as it
        # lands; it 4-7 follow kt-inner once gt is complete.
        pm03 = [psum2.tile([128, 256], F32, name=f"pm{i}", tag="mchain", bufs=4)
                for i in range(4)]
        for kp in range(4):
            pst = psum.tile([128, 512], BF16, name="pst", tag="ps", bufs=4)
            for j in range(2):
                kt = kp * 2 + j
                nc.tensor.transpose(pst[:, j * 256:j * 256 + 128],
                                    s0[:, kt * 128:(kt + 1) * 128], ident[:])
                nc.tensor.transpose(pst[:, j * 256 + 128:j * 256 + 256],
                                    s1t_[:, kt * 128:(kt + 1) * 128], ident[:])
            nc.scalar.copy(
                gt[:, kp * 2:kp * 2 + 2, :].rearrange("p a b -> p (a b)"), pst[:])
            for j in range(2):
                kt = kp * 2 + j
                for it in range(4):
                    nc.tensor.matmul(
                        pm03[it][:],
                        wqsb[:, kt, it * 128:(it + 1) * 128],
                        gt[:, kt, :],
                        start=(kt == 0),
                        stop=(kt == MT - 1),
                    )
        for it in range(4):
            nc.scalar.copy(msb[:, it, :], pm03[it][:])
        for it in range(4, MT):
            pm = psum2.tile([128, 256], F32, name="pm2", tag="mchain", bufs=4)
            for kt in range(MT):
                nc.tensor.matmul(
                    pm[:],
                    wqsb[:, kt, it * 128:(it + 1) * 128],
                    gt[:, kt, :],
                    start=(kt == 0),
                    stop=(kt == MT - 1),
                )
            nc.scalar.copy(msb[:, it, :], pm[:])

        # ---- Phase 8: y[:, js] = h~ @ M[:, js], 32 l-tiles (2 per bank) ----
        for lt in range(LT):
            py = psum.tile([128, 256], F32, name="py", tag="ps", bufs=4)
            for kt in range(MT):
                nc.tensor.matmul(
                    py[:],
                    hTr[:, kt, lt * 128:(lt + 1) * 128],
                    msb[:, kt, :],
                    start=(kt == 0),
                    stop=(kt == MT - 1),
                )
            yo = stream.tile([128, 256], BF16, name="yo", tag="yo", bufs=4)
            nc.scalar.copy(yo[:], py[:])
            nc.scalar.dma_start(y_ap[:, lt, :], yo[:])

        if dbg is not None:
            aps = {k: d.ap().rearrange("(t p) c -> p t c", p=128)
                   for k, d in dbg.items()}
            for t in range(NT):
                nc.sync.dma_start(aps["dbg_hr"][:, t, :], hr[:, t, :])
            for t in range(MT):
                nc.sync.dma_start(aps["dbg_htr"][:, t, :], hTr[:, t, :])
                nc.sync.dma_start(aps["dbg_gt"][:, t, :], gt[:, t, :])
                nc.sync.dma_start(aps["dbg_m"][:, t, :], msb[:, t, :])
            nc.sync.dma_start(aps["dbg_s"][:, 0, :], s0[:])
            nc.sync.dma_start(aps["dbg_s"][:, 1, :], s1t_[:])
            for t in range(NT):
                gbt = stream.tile([128, H], BF16, name="gbt", tag="gbt", bufs=2)
                nc.scalar.dma_start(gbt[:], gb_ap[:, t, :])
                nc.vector.tensor_copy(gbt[:], gbt[:])
                nc.sync.dma_start(aps["dbg_gb"][:, t, :], gbt[:])


_NC_CACHE = {}


def _build():
    if "nc" in _NC_CACHE:
        return _NC_CACHE["nc"]
    nc = bacc.Bacc("TRN2", target_bir_lowering=False, debug=False, num_devices=8)
    hn_d = nc.dram_tensor("hn", [LC, H], BF16, kind="ExternalInput")
    c1_d = nc.dram_tensor("c1", [LC, HH], BF16, kind="ExternalInput")
    s1_d = nc.dram_tensor("s1", [LC, HH], BF16, kind="ExternalInput")
    ht_d = nc.dram_tensor("ht", [H, L], BF16, kind="ExternalInput")
    ct_d = nc.dram_tensor("ct", [HH, L], BF16, kind="ExternalInput")
    st_d = nc.dram_tensor("st", [HH, L], BF16, kind="ExternalInput")
    wq_d = nc.dram_tensor("wq", [H, H], BF16, kind="ExternalInput")
    y_d = nc.dram_tensor("y", [L, JS], BF16, kind="ExternalOutput")
    with tile.TileContext(nc) as tc:
        _emit_once(nc, tc, hn_d, c1_d, s1_d, ht_d, ct_d, st_d, wq_d, y_d)
    nc.compile()
    _NC_CACHE["nc"] = nc
    return nc


def _in_maps(hidden_states, W_q, cos, sin):
    hs = np.asarray(hidden_states, dtype=np.float32)
    wq = np.asarray(W_q, dtype=np.float32).astype(BF16_NP)
    cos = np.asarray(cos, dtype=np.float32)
    sin = np.asarray(sin, dtype=np.float32)
    c1 = cos[:, :HH].astype(BF16_NP)
    s1 = sin[:, :HH].astype(BF16_NP)
    ct = np.ascontiguousarray(c1.T)
    st = np.ascontiguousarray(s1.T)
    hsb = hs.astype(BF16_NP)
    hts = [np.ascontiguousarray(hsb[b].T) for b in range(B)]
    maps = []
    for core in range(8):
        b, r = core // 4, core % 4
        sl = slice(r * LC, (r + 1) * LC)
        maps.append({
            "hn": np.ascontiguousarray(hsb[b, sl]),
            "c1": np.ascontiguousarray(c1[sl]),
            "s1": np.ascontiguousarray(s1[sl]),
            "ht": hts[b],
            "ct": ct,
            "st": st,
            "wq": wq,
        })
    return maps


def kernel(hidden_states, W_q, cos, sin):
    maps = _in_maps(hidden_states, W_q, cos, sin)
    nc = _build()
    res = run_bass_kernel_spmd(nc, maps, core_ids=list(range(8)))
    out = np.empty((B, L, H), dtype=np.float32)
    for core, r in enumerate(res.results):
        b, rr = core // 4, core % 4
        out[b, :, rr * JS:(rr + 1) * JS] = np.asarray(r["y"], dtype=np.float32)
    return out
